# revision 1
# baseline (speedup 1.0000x reference)
"""GCN encoder (2-layer) on 8 Trainium2 NeuronCores.

Row-parallel sharding: core r owns rows [r*1024, (r+1)*1024) of x / adjacency.

Math (reference):
    a = A + I;  d = rowsum(a)^-1/2;  a_norm = d[:,None] * a * d[None,:]
    h   = relu(a_norm @ (x @ w1.T + b1))
    out = a_norm @ (h @ w2.T + b2)

Device algorithm per core (no rank-dependence inside the NEFF):
    deg_r  = rowsum(A_r) + 1           (PE ones-matmul over A_r^T tiles)
    d_r    = rsqrt(deg_r)              (local slice of d; never all-gathered:
                                        each rank pre-scales its own support
                                        rows before the all-gather, so the
                                        gathered support is fully column-scaled)
    s1s_r  = d_r * (x_r @ w1.T + b1)   -> AllGather (split in 2) -> s1s (bf16)
    Q1^T   = s1s^T @ A_r^T + diag-term (identity contribution via small
                                        diagonal matmuls: + d_i * s1s[i,:])
    hhat^T = relu(Q1^T)                (h = d_r * hhat, folded downstream)
    s2s_r  = d_r^2 * (hhat_r @ w2.T) + d_r * b2  -> AllGather (split) -> s2s
    out^T  = d_r * (s2s^T @ A_r^T + diag-term)

Layer-1 aggregation runs i-chunk-outer so layer-2's support for chunk 0 (and
its AllGather) overlaps layer-1's chunk-1 matmuls. Both AllGathers are split
in row-halves so the first half's gather/load overlaps compute.

All big matmuls are bf16 x bf16 with fp32 PSUM accumulation.
A is cast to bf16 and pre-transposed host-side (layout prep only).
Output is produced transposed ([128, 1024] per core) and re-transposed host-side.
"""

import os
import sys

import numpy as np
import ml_dtypes

sys.path.insert(0, "/opt/trn_rl_repo")

BF16 = ml_dtypes.bfloat16

N, F_IN, F_HID, F_OUT = 8192, 512, 256, 128
NCORES = 8
NB = N // NCORES  # 1024 rows per core
P = 128
NT = NB // P      # 8 local row tiles
JT = N // P       # 64 j tiles
JG = 8            # j tiles per at-group  -> 8 groups of [128, 8, 1024] (2 MB DMAs)
NG = JT // JG     # 8
HC = NB // 2      # 512-wide i chunks
MT = F_HID // P   # 2 f1 tiles
KX = F_IN // P    # 4 xt tiles

_cached = {}


def _build_bass(reps=1):
    import concourse.bacc as bacc
    import concourse.tile as tile
    import concourse.mybir as mybir

    dt = mybir.dt

    nc = bacc.Bacc(
        "TRN2",
        target_bir_lowering=False,
        debug=False,
        enable_asserts=True,
        num_devices=NCORES,
    )

    # ---- kernel I/O ----
    at_d = nc.dram_tensor("at", [N, NB], dt.bfloat16, kind="ExternalInput")
    xt_d = nc.dram_tensor("xt", [F_IN, NB], dt.bfloat16, kind="ExternalInput")
    w1t_d = nc.dram_tensor("w1t", [F_IN, F_HID], dt.bfloat16, kind="ExternalInput")
    w2t_d = nc.dram_tensor("w2t", [F_HID, F_OUT], dt.bfloat16, kind="ExternalInput")
    b1_d = nc.dram_tensor("b1r", [1, F_HID], dt.float32, kind="ExternalInput")
    b2_d = nc.dram_tensor("b2r", [1, F_OUT], dt.float32, kind="ExternalInput")
    out_d = nc.dram_tensor("out_t", [F_OUT, NB], dt.float32, kind="ExternalOutput")

    # ---- internal DRAM ----
    # AllGathers are split in local-row halves: half h gathers every rank's
    # local rows [h*512, (h+1)*512) -> [8 ranks * 512, F] in rank-major order.
    ag1_in = [
        nc.dram_tensor(f"ag1{h}_in", [NB // 2, F_HID], dt.bfloat16, kind="Internal")
        for h in range(2)
    ]
    ag1_out = [
        nc.dram_tensor(
            f"ag1{h}_out", [N // 2, F_HID], dt.bfloat16, kind="Internal",
            addr_space="Shared",
        )
        for h in range(2)
    ]
    ag2_in = [
        nc.dram_tensor(f"ag2{h}_in", [NB // 2, F_OUT], dt.bfloat16, kind="Internal")
        for h in range(2)
    ]
    ag2_out = [
        nc.dram_tensor(
            f"ag2{h}_out", [N // 2, F_OUT], dt.bfloat16, kind="Internal",
            addr_space="Shared",
        )
        for h in range(2)
    ]
    ident_d = nc.inline_tensor(np.eye(P, dtype=BF16), name="ident128")

    rg = [list(range(NCORES))]
    io = dict(
        at_d=at_d, xt_d=xt_d, w1t_d=w1t_d, w2t_d=w2t_d, b1_d=b1_d, b2_d=b2_d,
        out_d=out_d, ag1_in=ag1_in, ag1_out=ag1_out, ag2_in=ag2_in,
        ag2_out=ag2_out, ident_d=ident_d, rg=rg,
    )

    with tile.TileContext(nc) as tc:
        with (
            tc.tile_pool(name="p_at", bufs=NG) as p_at,
            tc.tile_pool(name="p_sup", bufs=2) as p_sup,
            tc.tile_pool(name="p_misc", bufs=1) as p_misc,
            tc.tile_pool(name="p_ps_big", bufs=4, space="PSUM") as pp_big,
            tc.tile_pool(name="p_ps_small", bufs=2, space="PSUM") as pp_small,
            tc.tile_pool(name="p_ps_deg", bufs=2, space="PSUM") as pp_deg,
        ):
            pools = dict(
                p_at=p_at, p_sup=p_sup, p_misc=p_misc,
                pp_big=pp_big, pp_small=pp_small, pp_deg=pp_deg,
            )
            for _ in range(reps):
                _emit_body(nc, mybir, pools, io)

    nc.compile()
    return nc


def _emit_body(nc, mybir, pools, io):
    dt = mybir.dt
    AF = mybir.ActivationFunctionType
    p_at, p_sup, p_misc = pools["p_at"], pools["p_sup"], pools["p_misc"]
    pp_big, pp_small, pp_deg = pools["pp_big"], pools["pp_small"], pools["pp_deg"]
    at_d, xt_d, w1t_d, w2t_d = io["at_d"], io["xt_d"], io["w1t_d"], io["w2t_d"]
    b1_d, b2_d, out_d = io["b1_d"], io["b2_d"], io["out_d"]
    ag1_in, ag1_out, ag2_in, ag2_out = (
        io["ag1_in"], io["ag1_out"], io["ag2_in"], io["ag2_out"],
    )
    ident_d, rg = io["ident_d"], io["rg"]

    # ---- constants / weights into SBUF ----
    ones_col = p_misc.tile([P, 1], dt.bfloat16, tag="ones_col", name="ones_col")
    nc.gpsimd.memset(ones_col[:], 1.0)
    ones_row_f32 = p_misc.tile([1, P], dt.float32, tag="ones_row", name="ones_row")
    nc.gpsimd.memset(ones_row_f32[:], 1.0)

    w1t_sb = p_misc.tile([P, KX, F_HID], dt.bfloat16, tag="w1t", name="w1t_sb")
    nc.sync.dma_start(w1t_sb[:], w1t_d.ap().rearrange("(t p) f -> p t f", p=P))
    b1_sb = p_misc.tile([1, F_HID], dt.float32, tag="b1", name="b1_sb")
    nc.sync.dma_start(b1_sb[:], b1_d[:])

    xt_sb = []
    for k in range(KX):
        t = p_misc.tile([P, NB], dt.bfloat16, tag="hx", bufs=4, name="xt_sb")
        nc.sync.dma_start(t[:], xt_d[k * P : (k + 1) * P, :])
        xt_sb.append(t)

    # ---- A^T tiles (they gate deg -> everything): 7 groups of
    # [128, 8, 1024] plus a split last group so the rowsum tail is short ----
    at_tiles = []  # (tile, jt0, n_jt)
    for g in range(NG - 1):
        t = p_at.tile([P, JG, NB], dt.bfloat16, tag="at", bufs=NG - 1, name="at_sb")
        nc.sync.dma_start(
            t[:],
            at_d[g * JG * P : (g + 1) * JG * P, :].rearrange("(t p) i -> p t i", p=P),
        )
        at_tiles.append((t, g * JG, JG))
    for piece in range(4):
        jt0 = (NG - 1) * JG + piece * (JG // 4)
        t = p_at.tile([P, JG // 4, NB], dt.bfloat16, tag="at2", bufs=4, name="at2_sb")
        nc.sync.dma_start(
            t[:],
            at_d[jt0 * P : (jt0 + JG // 4) * P, :].rearrange("(t p) i -> p t i", p=P),
        )
        at_tiles.append((t, jt0, JG // 4))
    _at_lut = {}
    for t, jt0, n in at_tiles:
        for i in range(n):
            _at_lut[jt0 + i] = (t, i)

    # late-needed constants, off the at-load queue
    ident = p_misc.tile([P, P], dt.bfloat16, tag="ident", name="ident")
    nc.scalar.dma_start(ident[:], ident_d[:])
    w2t_sb = p_misc.tile([P, MT, F_OUT], dt.bfloat16, tag="w2t", name="w2t_sb")
    nc.scalar.dma_start(w2t_sb[:], w2t_d.ap().rearrange("(t p) f -> p t f", p=P))
    b2_sb = p_misc.tile([1, F_OUT], dt.float32, tag="b2", name="b2_sb")
    nc.scalar.dma_start(b2_sb[:], b2_d[:])

    def at_slice(jt, c):
        t, i = _at_lut[jt]
        return t[:, i, c * HC : (c + 1) * HC]

    # ---- layer-1 local support (pre-d): s1u = x @ w1.T + b1, bf16 staged ----
    s1u = p_misc.tile([P, NT, F_HID], dt.bfloat16, tag="s1u", name="s1u")
    for m in range(NT):
        ps = pp_small.tile([P, F_HID], dt.float32, tag="ps_small", name="ps_small")
        for k in range(KX):
            nc.tensor.matmul(
                ps[:], xt_sb[k][:, m * P : (m + 1) * P], w1t_sb[:, k, :],
                start=(k == 0), stop=False,
            )
        nc.tensor.matmul(ps[:], ones_row_f32[:], b1_sb[:], start=False, stop=True)
        nc.scalar.activation(s1u[:, m, :], ps[:], AF.Copy)

    # ---- rowsums via ones-matmul: deg[i] = sum_j A^T[j, i] ----
    deg_ps = [
        pp_deg.tile([1, HC], dt.float32, tag="deg", name="deg") for _ in range(2)
    ]
    for jt in range(JT):
        for c in range(2):
            nc.tensor.matmul(
                deg_ps[c][:],
                ones_col[:],
                at_slice(jt, c),
                start=(jt == 0),
                stop=(jt == JT - 1),
            )

    # ---- d = rsqrt(deg + 1) ----
    # fast path (critical): deg row -> PE-transpose -> [128, NT] per-partition
    # math; slow path ([1, NB] row layouts for d_bcast / layer-2 bias) follows
    # off the critical path.
    deg_sb = p_misc.tile([1, NB], dt.float32, tag="rowvec", bufs=2, name="deg_sb")
    degp = pp_small.tile([P, NT], dt.float32, tag="ps_small", name="degp")
    for c in range(2):
        nc.vector.tensor_scalar_add(
            deg_sb[:, c * HC : (c + 1) * HC], deg_ps[c][:], 1.0
        )
        for k in range(c * 4, c * 4 + 4):
            nc.tensor.transpose(
                degp[:, k : k + 1],
                deg_sb[0:1, k * P : (k + 1) * P],
                ones_row_f32[0:1, 0:1],
            )
    # dsq = 1/deg, dpart = sqrt(1/deg)
    dsq_part = p_misc.tile([P, NT], dt.float32, tag="dsq_part", name="dsq_part")
    _fast_recip = nc.vector.reciprocal(dsq_part[:], degp[:])
    dpart = p_misc.tile([P, NT], dt.float32, tag="dpart", name="dpart")
    nc.scalar.sqrt(dpart[:], dsq_part[:])

    # ---- post-d: scale s1 in place, ship halves to their AllGathers ----
    s1s = s1u
    _ship1 = None
    for h in range(2):
        for m in range(h * 4, h * 4 + 4):
            nc.vector.tensor_scalar_mul(
                s1s[:, m, :], s1u[:, m, :], dpart[:, m : m + 1]
            )
        _ship1 = nc.scalar.dma_start(
            ag1_in[h].ap().rearrange("(t p) f -> p t f", p=P),
            s1s[:, h * 4 : h * 4 + 4, :],
        )
        nc.gpsimd.collective_compute(
            "AllGather", mybir.AluOpType.bypass, replica_groups=rg,
            ins=[ag1_in[h].ap()], outs=[ag1_out[h].ap()],
        )

    # HAM warm-keeper: self-paced PE activity across the AllGather wait so
    # layer-1 starts at full clock. Each link: K=1 matmul (reads the previous
    # link's SBUF row) -> DVE copy back to SBUF. Values are unused.
    wsb = [
        p_misc.tile([1, F_HID], dt.float32, tag="warm", bufs=2, name="warm_sb")
        for _ in range(2)
    ]
    nc.vector.tensor_copy(wsb[0][:], s1s[0:1, 0, :])
    for link in range(12):
        wps = pp_deg.tile([1, F_HID], dt.float32, tag="deg", name="warm_ps")
        nc.tensor.matmul(
            wps[:], wsb[link % 2][0:1, 0:1], wsb[link % 2][:],
            start=True, stop=True,
        )
        nc.vector.tensor_copy(wsb[(link + 1) % 2][:], wps[:])

    # diagonal d tiles (bf16) for the identity contribution (needed only by
    # the layer-1 diag matmuls, after the j-loop)
    diag_sb = []
    for t in range(NT):
        dg = p_misc.tile([P, P], dt.bfloat16, tag="diag", bufs=NT, name="diag")
        nc.vector.tensor_scalar_mul(dg[:], ident[:], dpart[:, t : t + 1])
        diag_sb.append(dg)



    # gathered support halves, loaded in rank-pair quarters for pipelining
    sg1 = [[None] * 4 for _ in range(2)]
    for h in range(2):
        for q in range(4):
            t = p_sup.tile([P, 2, 4, F_HID], dt.bfloat16, tag="sup", bufs=8, name="sup")
            nc.scalar.dma_start(
                t[:],
                ag1_out[h][q * 1024 : (q + 1) * 1024, :].rearrange(
                    "(r s p) f -> p r s f", p=P, r=2
                ),
            )
            sg1[h][q] = t


    # slow path: [1, NB] row layouts (only needed from layer-2 on); explicitly
    # ordered behind the fast d-chain so it cannot steal the DVE first
    from concourse.tile_rust import add_dep_helper
    dinv_deg = p_misc.tile([1, NB], dt.float32, tag="rowvec", bufs=2, name="dinv_deg")
    _slow_recip = nc.vector.reciprocal(dinv_deg[:], deg_sb[:])
    add_dep_helper(_slow_recip.ins, _ship1.ins, sync=True,
                   reason="slow row-layout chain yields DVE until the AG1 ship")
    drow = p_misc.tile([1, NB], dt.float32, tag="drow", name="drow")
    nc.scalar.sqrt(drow[:], dinv_deg[:])
    # 1/d for the layer-2 bias matmul
    dinv_row = p_misc.tile([1, NB], dt.float32, tag="rowvec", bufs=2, name="dinv_row")
    nc.vector.reciprocal(dinv_row[:], drow[:])

    # d broadcast across partitions [128, 1024] via K=1 outer product
    d_bcast = p_misc.tile([P, NB], dt.float32, tag="d_bcast", name="d_bcast")
    for c in range(2):
        ps = pp_small.tile([P, HC], dt.float32, tag="ps_small", name="ps_small")
        nc.tensor.matmul(
            ps[:], ones_row_f32[:], drow[:, c * HC : (c + 1) * HC],
            start=True, stop=True,
        )
        nc.vector.tensor_copy(d_bcast[:, c * HC : (c + 1) * HC], ps[:])

    def sup_slice(sg, jt, f0, f1):
        r, lt = jt // NT, jt % NT
        h, sub = lt // 4, lt % 4
        return sg[h][r // 2][:, r % 2, sub, f0:f1]

    # j-tile visit order matches gather/load landing order: half, then quarter
    jorder = [
        r * NT + h * 4 + s
        for h in range(2)
        for q in range(4)
        for r in (2 * q, 2 * q + 1)
        for s in range(4)
    ]

    # ---- layer-1 aggregation, i-chunk-outer; layer-2 support per chunk ----
    hT = [
        p_misc.tile([P, NB], dt.bfloat16, tag="hx", bufs=4, name="hT")
        for _ in range(MT)
    ]
    s2s = p_misc.tile([P, NT, F_OUT], dt.bfloat16, tag="s2s", name="s2s")
    for c in range(2):
        q1 = [
            pp_big.tile([P, HC], dt.float32, tag="ps_big", name="ps_big")
            for _ in range(MT)
        ]
        # identity contribution first: the 4 slices cover the whole tile with
        # start=True, and they only need local data, so they run during the
        # AllGather window while the PE is otherwise idle.
        for m in range(MT):
            for k in range(4):
                t = c * 4 + k
                nc.tensor.matmul(
                    q1[m][:, k * P : (k + 1) * P],
                    s1s[:, t, m * P : (m + 1) * P],
                    diag_sb[t][:],
                    start=True,
                    stop=False,
                )
        for idx, jt in enumerate(jorder):
            for m in range(MT):
                nc.tensor.matmul(
                    q1[m][:],
                    sup_slice(sg1, jt, m * P, (m + 1) * P),
                    at_slice(jt, c),
                    start=False,
                    stop=(idx == len(jorder) - 1),
                )
        # hhat^T = relu(Q1^T), bf16
        for m in range(MT):
            nc.scalar.activation(hT[m][:, c * HC : (c + 1) * HC], q1[m][:], AF.Relu)

        # layer-2 local support for this chunk's rows:
        # s2s = d^2 * (hhat @ w2.T) + d * b2
        for m in range(c * 4, c * 4 + 4):
            ps = pp_small.tile([P, F_OUT], dt.float32, tag="ps_small", name="ps_small")
            for k in range(MT):
                nc.tensor.matmul(
                    ps[:], hT[k][:, m * P : (m + 1) * P], w2t_sb[:, k, :],
                    start=(k == 0), stop=False,
                )
            # bias: (1/d_i) * b2, so the d^2 epilogue scale leaves d_i * b2
            nc.tensor.matmul(
                ps[:], dinv_row[:, m * P : (m + 1) * P], b2_sb[:],
                start=False, stop=True,
            )
            nc.scalar.activation(
                s2s[:, m, :], ps[:], AF.Copy, scale=dsq_part[:, m : m + 1]
            )
        nc.scalar.dma_start(
            ag2_in[c].ap().rearrange("(t p) f -> p t f", p=P),
            s2s[:, c * 4 : c * 4 + 4, :],
        )
        nc.gpsimd.collective_compute(
            "AllGather", mybir.AluOpType.bypass, replica_groups=rg,
            ins=[ag2_in[c].ap()], outs=[ag2_out[c].ap()],
        )

    sg2 = [[None] * 4 for _ in range(2)]
    for h in range(2):
        for q in range(4):
            t = p_sup.tile([P, 2, 4, F_OUT], dt.bfloat16, tag="sup", bufs=8, name="sup")
            nc.scalar.dma_start(
                t[:],
                ag2_out[h][q * 1024 : (q + 1) * 1024, :].rearrange(
                    "(r s p) f -> p r s f", p=P, r=2
                ),
            )
            sg2[h][q] = t

    # ---- layer-2 aggregation + final scale ----
    for c in range(2):
        o_ps = pp_big.tile([P, HC], dt.float32, tag="ps_big", name="ps_big")
        for k in range(4):
            t = c * 4 + k
            nc.tensor.matmul(
                o_ps[:, k * P : (k + 1) * P],
                s2s[:, t, :],
                diag_sb[t][:],
                start=True,
                stop=False,
            )
        for idx, jt in enumerate(jorder):
            nc.tensor.matmul(
                o_ps[:],
                sup_slice(sg2, jt, 0, F_OUT),
                at_slice(jt, c),
                start=False,
                stop=(idx == len(jorder) - 1),
            )
        ot = p_misc.tile([P, HC], dt.float32, tag="hx", bufs=4, name="outT")
        nc.vector.tensor_tensor(
            ot[:], o_ps[:], d_bcast[:, c * HC : (c + 1) * HC],
            op=mybir.AluOpType.mult,
        )
        nc.scalar.dma_start(out_d[:, c * HC : (c + 1) * HC], ot[:])


def make_in_maps(x, A, w1, b1, w2, b2):
    w1t = np.ascontiguousarray(w1.T).astype(BF16)
    w2t = np.ascontiguousarray(w2.T).astype(BF16)
    b1r = np.ascontiguousarray(b1[None, :]).astype(np.float32)
    b2r = np.ascontiguousarray(b2[None, :]).astype(np.float32)
    in_maps = []
    for r in range(NCORES):
        rows = slice(r * NB, (r + 1) * NB)
        in_maps.append(
            {
                "at": np.ascontiguousarray(A[rows, :].T.astype(BF16)),
                "xt": np.ascontiguousarray(x[rows, :].T.astype(BF16)),
                "w1t": w1t,
                "w2t": w2t,
                "b1r": b1r,
                "b2r": b2r,
            }
        )
    return in_maps


def kernel(x, adjacency_matrix, w1, b1, w2, b2):
    from concourse.bass_utils import run_bass_kernel_spmd

    x = np.asarray(x, dtype=np.float32)
    A = np.asarray(adjacency_matrix, dtype=np.float32)
    w1 = np.asarray(w1, dtype=np.float32)
    b1 = np.asarray(b1, dtype=np.float32)
    w2 = np.asarray(w2, dtype=np.float32)
    b2 = np.asarray(b2, dtype=np.float32)

    if "nc" not in _cached:
        _cached["nc"] = _build_bass()
    nc = _cached["nc"]

    in_maps = make_in_maps(x, A, w1, b1, w2, b2)
    res = run_bass_kernel_spmd(nc, in_maps, core_ids=list(range(NCORES)))

    out = np.empty((N, F_OUT), dtype=np.float32)
    for r in range(NCORES):
        out[r * NB : (r + 1) * NB, :] = res.results[r]["out_t"].T
    return out

